# revision 21
# baseline (speedup 1.0000x reference)
"""Trainium2 Bass kernel for nn_MultiHeadDilatedState.

Sharding: data-parallel over batch (B=8 -> 8 cores, one sequence per core).
Weights replicated. Per-core dataflow is channel-major [768, 4096] fp16:

  x [S,H] --PE transpose--> xt fp16 [128,6,S] --fp16 matmuls--> GLU + router
  conv: 3 stages ping-pong A->B->A->B.  Work is split across engines:
    - (j0,c0),(j0,c1) on PE as fp16 diag matmuls (quadrant tile_position)
      with DVE STT evacuation (residual+bias fused),
    - everything else restructured as  hnew = (1+w3)*h + b  (tap0, on ACT
      with per-partition scale/bias)  then per shifted tap m: partial
      p = w_{3-m} * h  (tensor_scalar on ACT or DVE, 4x mode)  and
      hnew[m*d:] += p  (tensor_tensor on DVE, 2x mode).
  head-weight gating via one-hot replication matmul + DVE mult,
  mix-gate fp16 matmuls, final matmul token-major (activation stationary).
"""

import os
import numpy as np

import concourse.bass as bass
import concourse.bacc as bacc
import concourse.mybir as mybir
import concourse.tile as tile
from concourse.bass_utils import run_bass_kernel_spmd
from concourse.masks import make_identity

B, S, HID = 8, 4096, 768
NH, HD, KT = 12, 64, 4
NC = 6                  # 768 / 128 channel chunks
ST = 512                # token tile (one PSUM bank of fp32)
NST = S // ST           # 8
F32 = mybir.dt.float32
F16 = mybir.dt.float16
F8 = mybir.dt.float8e4
DR = mybir.MatmulPerfMode.DoubleRow
SIG = mybir.ActivationFunctionType.Sigmoid
IDENT = mybir.ActivationFunctionType.Identity
ADD = mybir.AluOpType.add
MUL = mybir.AluOpType.mult

DILATIONS = [(1, 2, 4), (1, 1, 1), (4, 8, 16), (8, 16, 32), (32, 64, 128),
             (64, 128, 256), (256, 512, 1024), (1, 100, 200), (1, 500, 1000),
             (1, 1024, 2048), (3, 9, 27), (5, 25, 125)]

# conv (stage, chunk) pairs executed on the PE as diag matmuls
PE_PAIRS = [(0, 0), (1, 0), (2, 0), (0, 1), (1, 1), (2, 1), (2, 2), (2, 5)]
# (stage, chunk) pairs whose shifted-tap partials run on DVE instead of ACT
DVE_TS = set()
# GLU emission order: chunks with long off-PE conv chains first
OC_ORDER = [3, 5, 2, 4, 1, 0]


def build_bass():
    nc = bacc.Bacc()

    x_d = nc.dram_tensor("xb", [S, HID], F32, kind="ExternalInput")
    gwT_d = nc.dram_tensor("gwT16", [128, NC, 2 * HID], F16, kind="ExternalInput")
    mgmix_d = nc.dram_tensor("mgmix16", [128, NC, 2 * HID], F16,
                             kind="ExternalInput")
    mgT8_d = nc.dram_tensor("mgT8", [128, NC, HID], F8, kind="ExternalInput")
    rwT_d = nc.dram_tensor("rwT16", [128, NC, 32], F16, kind="ExternalInput")
    rb_d = nc.dram_tensor("rb", [NH, 1], F32, kind="ExternalInput")
    cvd_d = nc.dram_tensor("cvd16", [128, len(PE_PAIRS), 256], F16,
                           kind="ExternalInput")
    convsc_d = nc.dram_tensor("convsc", [128, 20, 8], F32, kind="ExternalInput")
    convb_d = nc.dram_tensor("convb", [128, 32], F32, kind="ExternalInput")
    erep_d = nc.dram_tensor("erep16", [NH, NC, 128], F16, kind="ExternalInput")
    mgb_d = nc.dram_tensor("mgb", [128, 32], F32, kind="ExternalInput")
    mixbias_d = nc.dram_tensor("mixbias", [128, HID], F16, kind="ExternalInput")
    out_d = nc.dram_tensor("out", [S, HID], F16, kind="ExternalOutput")
    dbg_d = (nc.dram_tensor("dbg", [NC, 128, S], F32, kind="ExternalOutput")
             if os.environ.get("KDBG") else None)

    with tile.TileContext(nc) as tc:
        _body(tc, x_d, gwT_d, mgmix_d, mgT8_d, rwT_d, rb_d, cvd_d, convsc_d,
              convb_d, erep_d, mgb_d, mixbias_d, out_d, dbg_d)
    nc.finalize()
    return nc


def _body(tc, x_d, gwT_d, mgmix_d, mgT8_d, rwT_d, rb_d, cvd_d, convsc_d,
          convb_d, erep_d, mgb_d, mixbias_d, out_d, dbg_d=None):
    nc = tc.nc

    with (
        tc.tile_pool(name="persist", bufs=1) as persist,
        tc.tile_pool(name="xload", bufs=2) as p_xload,
        tc.tile_pool(name="sig", bufs=3) as p_sig,
        tc.tile_pool(name="part", bufs=2) as p_part,
    ):
        # ---- persistent tiles (128B-multiple free sizes, big first) ----
        hA = persist.tile([128, NC, S], F16, tag="hA", name="hA")
        hB = persist.tile([128, NC, S], F16, tag="hB", name="hB")
        xt = persist.tile([128, NC, S], F16, tag="xt", name="xt")
        gwT = persist.tile([128, NC, 2 * HID], F16, tag="gwT")
        nc.sync.dma_start(gwT, gwT_d[:, :, :])
        rwT_p = persist.tile([128, NC, 32], F16, tag="rwT")
        nc.sync.dma_start(rwT_p, rwT_d[:, :, :])
        rwT = rwT_p[:, :, 0:NH]
        convsc = persist.tile([128, 20, 8], F32, tag="convsc")
        nc.sync.dma_start(convsc, convsc_d[:, :, :])
        convb = persist.tile([128, 32], F32, tag="convb")
        nc.sync.dma_start(convb, convb_d[:, :])
        rb_p = persist.tile([NH, 32], F32, tag="rb")
        rb = rb_p[:, 0:1]
        nc.sync.dma_start(rb, rb_d[:, :])
        cvd16 = persist.tile([128, len(PE_PAIRS), 256], F16, tag="cvd16")
        nc.sync.dma_start(cvd16, cvd_d[:, :, :])
        ident = persist.tile([128, 128], F32, tag="ident")
        make_identity(nc, ident[:, :])
        hws = persist.tile([NH, S], F16, tag="hws")
        # late-phase weights
        erep = persist.tile([NH, NC, 128], F16, tag="erep")
        mgb_p = persist.tile([128, 32], F32, tag="mgb")
        mgb = mgb_p[:, 0:NC]
        mixbias = persist.tile([128, HID], F16, tag="mixbias")

        def conv_dve(j, c):
            """Conv stage j for chunk c on ACT(+DVE): tap0 via activation,
            shifted taps as tensor_scalar partial + tensor_tensor add."""
            jc = j * NC + c
            src = hA if j != 1 else hB
            dst = hB if j != 1 else hA
            # tap0 + bias + residual scale, both heads at once (idle Pool)
            nc.gpsimd.tensor_scalar(dst[:, c, :], src[:, c, :],
                                    convsc[:, jc, 0:1], convb[:, jc:jc + 1],
                                    MUL, ADD)
            for half in (0, 1):
                h = 2 * c + half
                ho = 64 * half
                d = DILATIONS[h][j]
                ts_eng = nc.vector if (j, c) in DVE_TS else nc.scalar
                tt_eng = nc.vector
                for m in (1, 2, 3):
                    L = S - m * d
                    if L <= 0:
                        continue
                    part = p_part.tile([128, S], F16, tag="part")
                    if ts_eng is nc.scalar:
                        nc.scalar.activation(
                            part[ho:ho + 64, 0:L], src[ho:ho + 64, c, 0:L],
                            IDENT, scale=convsc[ho:ho + 64, jc, m:m + 1])
                    else:
                        nc.vector.tensor_scalar_mul(
                            part[ho:ho + 64, 0:L], src[ho:ho + 64, c, 0:L],
                            convsc[ho:ho + 64, jc, m:m + 1])
                    tt_eng.tensor_add(dst[ho:ho + 64, c, m * d:S],
                                      dst[ho:ho + 64, c, m * d:S],
                                      part[ho:ho + 64, 0:L])

        def conv_pe(j, c, psB):
            """Conv stage j for chunk c on PE: fp16 diag matmuls into PSUM,
            two heads concurrently via quadrant tile_position; DVE STT
            evacuation fuses residual + bias."""
            pi = PE_PAIRS.index((j, c))
            jc = j * NC + c
            src = hA if j != 1 else hB
            dst = hB if j != 1 else hA
            for st in range(NST):
                s0 = st * ST
                pc = psB.tile([128, ST], F32, tag="conv", bufs=2)
                mms = []
                for half in (0, 1):
                    p0 = 64 * half
                    d = DILATIONS[2 * c + half][j]
                    first = True
                    for m in range(KT):
                        off = m * d
                        if off >= s0 + ST:
                            continue
                        a = max(0, off - s0)
                        mms.append((p0, m, a, s0 - off + a, first))
                        first = False
                ev = [t for t in mms if t[0] == 0]
                od = [t for t in mms if t[0] == 64]
                mms = []
                for i in range(max(len(ev), len(od))):
                    if i < len(ev):
                        mms.append(ev[i])
                    if i < len(od):
                        mms.append(od[i])
                nlast = {0: None, 64: None}
                for i, (p0, m, a, r0, fi) in enumerate(mms):
                    nlast[p0] = i
                for i, (p0, m, a, r0, fi) in enumerate(mms):
                    nc.tensor.matmul(
                        pc[p0:p0 + 64, a:ST],
                        cvd16[p0:p0 + 64, pi, m * 64:(m + 1) * 64],
                        src[p0:p0 + 64, c, r0:r0 + ST - a],
                        start=fi, stop=(i == nlast[p0]),
                        tile_position=(p0, p0))
                nc.vector.scalar_tensor_tensor(
                    dst[:, c, s0:s0 + ST], pc[:, :], convb[:, jc:jc + 1],
                    src[:, c, s0:s0 + ST], op0=ADD, op1=ADD)

        # ============ phase A1: transpose + router (own PSUM scope) =======
        with tc.tile_pool(name="psT", bufs=1, space="PSUM") as psT:
            xs_tiles = []
            for st in range(NST):
                for sub in range(4):
                    xs = p_xload.tile([128, HID], F32, tag="xs")
                    nc.sync.dma_start(
                        xs, x_d[st * ST + sub * 128: st * ST + (sub + 1) * 128, :])
                    xs_tiles.append(xs)
            # late weight loads (queued behind x tiles on the DMA rings)
            nc.sync.dma_start(erep, erep_d[:, :, :])
            nc.sync.dma_start(mgb_p, mgb_d[:, :])
            nc.sync.dma_start(mixbias, mixbias_d[:, :])

            for st in range(NST):
                s0 = st * ST
                ptp = [psT.tile([128, ST], F32, tag=f"tp{kc}", bufs=1,
                                name=f"ptp{kc}")
                       for kc in range(NC)]
                for sub in range(4):
                    for kc in range(NC):
                        nc.tensor.transpose(
                            ptp[kc][:, sub * 128:(sub + 1) * 128],
                            xs_tiles[st * 4 + sub][:, kc * 128:(kc + 1) * 128],
                            ident[:, :])
                for kc in range(NC):
                    nc.vector.tensor_copy(xt[:, kc, s0:s0 + ST], ptp[kc][:, :])
                # router
                pr_t = psT.tile([128, ST], F32, tag="rtr", bufs=1)
                pr = pr_t[0:NH, :]
                for kc in range(NC):
                    nc.tensor.matmul(pr[:, :], rwT[:, kc, :],
                                     xt[:, kc, s0:s0 + ST],
                                     start=(kc == 0), stop=(kc == NC - 1))
                nc.scalar.activation(hws[:, s0:s0 + ST], pr[:, :], SIG,
                                     bias=rb[:, :], scale=1.0)

        # ================= phase A2: GLU + conv + gating ==================
        with (
            tc.tile_pool(name="psG", bufs=1, space="PSUM") as psG,
            tc.tile_pool(name="psB", bufs=1, space="PSUM") as psB,
        ):
            # GLU, output-chunk-major so convs can start per chunk;
            # chunks with the longest off-PE conv chains first
            for oc in OC_ORDER:
                for st in range(NST):
                    s0 = st * ST
                    pg = psG.tile([128, ST], F32, tag="glu", bufs=3)
                    for kc in range(NC):
                        nc.tensor.matmul(
                            pg[:, :],
                            gwT[:, kc, HID + oc * 128: HID + (oc + 1) * 128],
                            xt[:, kc, s0:s0 + ST],
                            start=(kc == 0), stop=(kc == NC - 1))
                    sg = p_sig.tile([128, ST], F16, tag="sig")
                    nc.scalar.activation(sg[:, :], pg[:, :], SIG)
                    pv = psG.tile([128, ST], F32, tag="glv", bufs=2)
                    for kc in range(NC):
                        nc.tensor.matmul(
                            pv[:, :],
                            gwT[:, kc, oc * 128:(oc + 1) * 128],
                            xt[:, kc, s0:s0 + ST],
                            start=(kc == 0), stop=(kc == NC - 1))
                    nc.vector.tensor_mul(hA[:, oc, s0:s0 + ST], pv[:, :],
                                         sg[:, :])
                # stage-0 conv for this chunk
                if (0, oc) in PE_PAIRS:
                    conv_pe(0, oc, psB)
                else:
                    conv_dve(0, oc)

            if dbg_d is not None and os.environ.get("KDBG") == "A":
                for c in range(NC):
                    nc.sync.dma_start(dbg_d[c, :, :], hA[:, c, :])

            # ---- conv stages 1, 2 ----
            for j in (1, 2):
                for c in range(NC):
                    if (j, c) in PE_PAIRS:
                        conv_pe(j, c, psB)
                    else:
                        conv_dve(j, c)

            # ---- head-weight gating: replicate [12,S] -> [128,S] ----
            for c in range(NC):
                for st in range(NST):
                    s0 = st * ST
                    ph = psB.tile([128, ST], F32, tag="hwr", bufs=1)
                    nc.tensor.matmul(ph[:, :], erep[:, c, :],
                                     hws[:, s0:s0 + ST], start=True, stop=True)
                    nc.vector.tensor_mul(hB[:, c, s0:s0 + ST],
                                         hB[:, c, s0:s0 + ST], ph[:, :])

            if dbg_d is not None and os.environ.get("KDBG") == "B":
                for c in range(NC):
                    nc.sync.dma_start(dbg_d[c, :, :], hB[:, c, :])

        # mix weights reuse gwT's space (gwT dead after GLU)
        mgmix = persist.tile([128, NC, 2 * HID], F16, tag="gwT", name="mgmix")
        nc.sync.dma_start(mgmix, mgmix_d[:, :, :])
        # fp8 mix-gate weights live in the (unused) first half of mgmix
        mgT8 = mgmix[:, :, 0:HID // 2].bitcast(F8)
        nc.sync.dma_start(mgT8, mgT8_d[:, :, :])
        mixT = mgmix[:, :, HID:2 * HID]

        # ============== phase C: mix gate;  phase D: final matmul =========
        with tc.tile_pool(name="psC", bufs=1, space="PSUM") as psC:
            o16 = persist.tile([128, NC, S], F16, tag="xt", name="o16")
            for st in range(NST):
                s0 = st * ST
                h8 = p_sig.tile([128, NC, ST], F8, tag="h8", bufs=1, name="h8")
                for kc in range(NC):
                    nc.vector.tensor_copy(h8[:, kc, :], hB[:, kc, s0:s0 + ST])
                for oc in range(NC):
                    pm = psC.tile([128, ST], F32, tag="mg", bufs=3)
                    for kp in range(NC // 2):
                        nc.tensor.matmul(
                            pm[:, :],
                            mgT8[:, 2 * kp:2 * kp + 2, oc * 128:(oc + 1) * 128],
                            h8[:, 2 * kp:2 * kp + 2, :],
                            start=(kp == 0), stop=(kp == NC // 2 - 1),
                            perf_mode=DR)
                    sg = p_sig.tile([128, ST], F16, tag="sig")
                    nc.scalar.activation(sg[:, :], pm[:, :], SIG,
                                         bias=mgb[:, oc:oc + 1], scale=1.0)
                    nc.vector.tensor_mul(o16[:, oc, s0:s0 + ST],
                                         hB[:, oc, s0:s0 + ST], sg[:, :])
                for tl in range(4):
                    c0 = st * ST + tl * 128
                    pmx = psC.tile([128, HID], F32, tag="mx", bufs=2)
                    for kc in range(NC):
                        nc.tensor.matmul(pmx[:, 0:512],
                                         o16[:, kc, c0:c0 + 128],
                                         mixT[:, kc, 0:512],
                                         start=(kc == 0), stop=(kc == NC - 1))
                    for kc in range(NC):
                        nc.tensor.matmul(pmx[:, 512:HID],
                                         o16[:, kc, c0:c0 + 128],
                                         mixT[:, kc, 512:HID],
                                         start=(kc == 0), stop=(kc == NC - 1))
                    osb_t = p_part.tile([128, S], F16, tag="part", name="osb")
                    osb = osb_t[:, 0:HID]
                    nc.vector.tensor_add(osb[:, :], pmx[:, :], mixbias[:, :])
                    nc.sync.dma_start(out_d[c0:c0 + 128, :], osb[:, :])


def _prep_weights(gate_w, conv_w, conv_b, router_w, router_b,
                  mix_gate_w, mix_gate_b, mixing_w, mixing_b):
    f = np.float32
    h = np.float16
    gwT16 = np.ascontiguousarray(
        gate_w.T.reshape(NC, 128, 2 * HID).transpose(1, 0, 2), dtype=h)
    mgmix16 = np.ascontiguousarray(
        np.concatenate([mix_gate_w.T, mixing_w.T], axis=1)
        .reshape(NC, 128, 2 * HID).transpose(1, 0, 2), dtype=h)
    f8 = mybir.dt.np(mybir.dt.float8e4)
    mgT8 = np.ascontiguousarray(
        mix_gate_w.T.reshape(NC, 128, HID).transpose(1, 0, 2)).astype(f8)
    rwT16 = np.zeros((128, NC, 32), dtype=h)
    rwT16[:, :, 0:NH] = router_w.T.reshape(NC, 128, NH).transpose(1, 0, 2)
    rb = np.ascontiguousarray(router_b.reshape(NH, 1), dtype=f)

    # fp16 tap diagonals for the PE-assigned pairs
    cd = np.zeros((128, len(PE_PAIRS), 256), dtype=h)
    ar = np.arange(HD)
    for pi, (j, c) in enumerate(PE_PAIRS):
        for half in (0, 1):
            hd = 2 * c + half
            for m in range(KT):
                w = conv_w[hd, j, :, KT - 1 - m].astype(h)
                cd[half * HD + ar, pi, m * HD + ar] = w

    # per-partition conv scalars: [:, jc, 0] = 1 + w3 (tap0 + residual),
    # [:, jc, m] = w_{3-m} for shifted tap m
    convsc = np.zeros((128, 20, 8), dtype=f)
    convb = np.zeros((128, 32), dtype=f)
    for j in range(3):
        for c in range(NC):
            jc = j * NC + c
            for half in (0, 1):
                hd = 2 * c + half
                sl = slice(half * HD, (half + 1) * HD)
                convsc[sl, jc, 0] = 1.0 + conv_w[hd, j, :, 3]
                for m in (1, 2, 3):
                    convsc[sl, jc, m] = conv_w[hd, j, :, 3 - m]
                convb[sl, jc] = conv_b[hd, j]

    erep16 = np.zeros((NH, NC, 128), dtype=h)
    for c in range(NC):
        for q in range(128):
            erep16[2 * c + (q >= HD), c, q] = 1.0

    mgb = np.zeros((128, 32), dtype=f)
    mgb[:, 0:NC] = mix_gate_b.reshape(NC, 128).T
    mixbias = np.ascontiguousarray(np.tile(mixing_b[None, :], (128, 1)),
                                   dtype=h)

    return {"gwT16": gwT16, "mgmix16": mgmix16, "mgT8": mgT8,
            "rwT16": np.ascontiguousarray(rwT16), "rb": rb,
            "cvd16": np.ascontiguousarray(cd),
            "convsc": convsc, "convb": convb,
            "erep16": erep16, "mgb": mgb, "mixbias": mixbias}


_CACHE = {}


def _run(inputs, trace=False, tmpdir=None):
    if "nc" not in _CACHE:
        _CACHE["nc"] = build_bass()
    nc = _CACHE["nc"]

    w = _prep_weights(
        np.asarray(inputs["gate_w"]), np.asarray(inputs["conv_w"]),
        np.asarray(inputs["conv_b"]), np.asarray(inputs["router_w"]),
        np.asarray(inputs["router_b"]), np.asarray(inputs["mix_gate_w"]),
        np.asarray(inputs["mix_gate_b"]), np.asarray(inputs["mixing_w"]),
        np.asarray(inputs["mixing_b"]))
    x = np.ascontiguousarray(np.asarray(inputs["x"]), dtype=np.float32)

    in_maps = [dict(w, xb=np.ascontiguousarray(x[b])) for b in range(B)]
    res = run_bass_kernel_spmd(nc, in_maps, core_ids=list(range(B)),
                               trace=trace, tmpdir=tmpdir)
    out = np.stack([res.results[b]["out"] for b in range(B)], axis=0)
    return out.astype(np.float32), res


def kernel(**inputs):
    out, _ = _run(inputs, trace=False)
    return out


if __name__ == "__main__":
    nc = build_bass()
    print("built ok; instructions:", len(nc.inst_map))
